# revision 16
# baseline (speedup 1.0000x reference)
"""CAM (channel attention module) Trainium2 kernel.

Reference computation (per sample b):
    xf = x[b].reshape(C, N)
    energy = xf @ xf.T                      # [C, C]
    att = softmax(max_row(energy) - energy) # row-wise == softmax(-energy)
    out = gamma * (att @ xf) + xf

Full shapes: x [128, 3, 16, 112, 112] f32, gamma [1] f32.
Data-parallel over batch: 16 samples per core on 8 NeuronCores.

v5 design (per core, 16 samples, streaming):
 - input DMA casts f32->bf16 in the SWDGE (gpsimd) path; SBUF holds bf16.
 - gram on the (otherwise idle) TensorE: for each of the 6 channel pairs,
   13 accumulating [128, <=128]^T @ [128, <=128] matmuls into a
   [128, 6, 128] PSUM tile (walrus requires single-free-dim matmul APs).
   Energies = PSUM diagonals, extracted by 6 DVE STT+accum ops against an
   identity mask, partition-reduced by a ones matmul + W2 gather (tiny).
 - apply (out_c = sum_d mb[c,d] x_d, mb = I + gamma*att) as 9 bf16
   tensor_scalar muls (split DVE/ScalarE; stock TS has 2x/4x uops) and
   2 channel-fused [P, 3F] bf16 tensor_tensor adds on DVE (2x mode).
   scalar_tensor_tensor is avoided for big ops: it only has a 1x uop.
 - output written bf16 (tolerance 2e-2; bf16 round-trip ~2e-3), host
   upconverts to f32.
 - no gpsimd compute (GpSimd and DVE's 2nd port share an exclusive SBUF
   port pair; mixing them serializes both engines).
"""

import sys

sys.path.insert(0, "/opt/trn_rl_repo")

import numpy as np

import concourse.bass as bass
import concourse.tile as tile
from concourse import mybir
from concourse.bass_utils import run_bass_kernel_spmd

B, C, T, H, W = 128, 3, 16, 112, 112
N = T * H * W                 # 200704
P = 128
F = N // P                    # 1568
NCORES = 8
S = B // NCORES               # 16 samples per core

GCH = 64                      # f-chunk per gram matmul
PAIRS6 = [(0, 0), (1, 1), (2, 2), (0, 1), (0, 2), (1, 2)]

FP32 = mybir.dt.float32
BF16 = mybir.dt.bfloat16
AX = mybir.AxisListType
ALU = mybir.AluOpType
ACT = mybir.ActivationFunctionType

# --- tuning knobs -----------------------------------------------------------
CFG = dict(
    swdge_in=True,   # cast f32->bf16 inside the input DMA (gpsimd SWDGE)
    ahead=6,         # gram/extract run this many samples ahead of apply
    in_bufs=9,       # lookahead = in_bufs - 1 samples
    out_bufs=3,
    gram_bufs=2,     # PSUM double-buffering for gram tiles
    group=4,         # samples per batched softmax chain
)
# apply-mul split: ScalarE gets these (d, c) pairs, DVE the rest
MULS_SCALAR = [(0, 0), (0, 1), (0, 2), (1, 0), (1, 1)]
MULS_DVE = [(1, 2), (2, 0), (2, 1), (2, 2)]


def _bcast(ap, n, pos):
    """Insert a 0-stride dim of extent n at position pos of the ap list."""
    new = list(ap.ap)
    new.insert(pos, [0, n])
    return bass.AP(tensor=ap.tensor, offset=ap.offset, ap=new)


def split_multi_waits(nc):
    """This container's walrus accepts only one sync-wait per instruction.
    Hoist extra waits onto single-wait NOPs on the same (in-order) queue."""
    n_split = 0
    for bb in nc.main_func.blocks:
        insts = list(bb.instructions)
        new = []
        for inst in insts:
            si = inst.sync_info
            waits = list(si.on_wait) if si is not None else []
            if len(waits) > 1:
                for i, w in enumerate(waits[:-1]):
                    nop = mybir.InstNoOp(
                        name=f"{inst.name}-wsplit{i}",
                        opcode="NoOp",
                        engine=inst.engine,
                        text_hint="wait_split",
                        bass_nofuse=True,
                        sync_info=mybir.SyncInfo(on_wait=[w], on_update=[]),
                    )
                    new.append(nop)
                    n_split += 1
                inst.sync_info = mybir.SyncInfo(
                    on_wait=[waits[-1]], on_update=list(si.on_update)
                )
            new.append(inst)
        if len(new) != len(insts):
            try:
                bb.instructions = new
            except Exception:
                del bb.instructions[:]
                bb.instructions.extend(new)
    return n_split


def build_kernel(cfg=CFG, s_per_core=S, n_free=F, split_waits=True):
    """Emit the per-core Tile program. DRAM views: [S, C, P, F]."""
    from contextlib import ExitStack

    nc = bass.Bass("TRN2", target_bir_lowering=False, debug=False)
    f = n_free
    # gram chunking: 12 full 128-wide chunks + one 32-wide remainder
    chunks = []
    pos = 0
    while pos < f:
        w = min(GCH, f - pos)
        chunks.append((pos, w))
        pos += w

    x_d = nc.dram_tensor("x", [s_per_core, C, P, f], FP32, kind="ExternalInput")
    g_d = nc.dram_tensor("gamma", [1, 1], FP32, kind="ExternalInput")
    i9_d = nc.dram_tensor("i9c", [1, 9], FP32, kind="ExternalInput")
    dg_d = nc.dram_tensor("diagm", [P, P], FP32, kind="ExternalInput")
    w2g_d = nc.dram_tensor(
        "w2g", [6 * cfg["group"], 9 * cfg["group"]], FP32, kind="ExternalInput"
    )
    i9g_d = nc.dram_tensor("i9g", [1, 9 * cfg["group"]], FP32, kind="ExternalInput")
    o_d = nc.dram_tensor("out", [s_per_core, C, P, f], BF16, kind="ExternalOutput")

    with tile.TileContext(nc) as tc, ExitStack() as ctx:
        consts = ctx.enter_context(tc.tile_pool(name="consts", bufs=1))
        in_pool = ctx.enter_context(tc.tile_pool(name="in", bufs=cfg["in_bufs"]))
        out_pool = ctx.enter_context(tc.tile_pool(name="outp", bufs=cfg["out_bufs"]))
        u_pool = ctx.enter_context(tc.tile_pool(name="u", bufs=2))
        small = ctx.enter_context(tc.tile_pool(name="small", bufs=4))
        psum = ctx.enter_context(tc.tile_pool(name="psum", bufs=1, space="PSUM"))
        gpsum = ctx.enter_context(
            tc.tile_pool(name="gpsum", bufs=cfg["gram_bufs"], space="PSUM")
        )

        # ---- constants ----
        ones_k = consts.tile([P, 1], FP32)          # partition-reduce rhs
        nc.vector.memset(ones_k, 1.0)
        ones_b = consts.tile([1, P], FP32)          # K=1 broadcast lhsT
        nc.vector.memset(ones_b, 1.0)
        i9 = consts.tile([1, 9], FP32)              # flat 3x3 identity
        nc.sync.dma_start(out=i9, in_=i9_d.ap())
        w2g = consts.tile([6 * CFG["group"], 9 * CFG["group"]], FP32)
        nc.sync.dma_start(out=w2g, in_=w2g_d.ap())
        i9g = consts.tile([1, 9 * CFG["group"]], FP32)
        nc.sync.dma_start(out=i9g, in_=i9g_d.ap())
        diagm = consts.tile([P, P], FP32)           # 128x128 identity mask
        nc.sync.dma_start(out=diagm, in_=dg_d.ap())
        gamma_sb = consts.tile([1, 1], FP32)
        nc.sync.dma_start(out=gamma_sb, in_=g_d.ap())

        xin_tiles = {}
        gram_tiles = {}
        mb_tiles = {}

        def emit_load(si):
            xin = in_pool.tile([P, C, f], BF16, tag="xin")
            src = x_d.ap()[si].rearrange("c p f -> p c f")
            if cfg["swdge_in"]:
                nc.gpsimd.dma_start(out=xin, in_=src)
            else:
                nc.sync.dma_start(out=xin, in_=src)
            xin_tiles[si] = xin

        def emit_gram(si):
            """TensorE: M[j][f,f'] = sum_n-chunks x_c[:,f] . x_d[:,f'] per pair."""
            xin = xin_tiles[si]
            m_ps = gpsum.tile([GCH, 6, GCH], FP32, tag="gram")
            for k, (p0, w) in enumerate(chunks):
                for j, (a, b) in enumerate(PAIRS6):
                    nc.tensor.matmul(
                        out=m_ps[:w, j, :w],
                        lhsT=xin[:, a, p0 : p0 + w],
                        rhs=xin[:, b, p0 : p0 + w],
                        start=(k == 0),
                        stop=(k == len(chunks) - 1),
                        skip_group_check=True,
                    )
            gram_tiles[si] = m_ps

        def emit_extract(si, dsum, slot):
            """diag sums of the 6 PSUM pair blocks -> dsum[:, 6*slot:6*slot+6]."""
            m_ps = gram_tiles[si]
            scr = small.tile([GCH, GCH], BF16, tag="scr")
            for j in range(6):
                nc.vector.scalar_tensor_tensor(
                    out=scr,
                    in0=m_ps[:GCH, j, :],
                    scalar=1.0,
                    in1=diagm[:GCH, :GCH],
                    op0=ALU.mult,
                    op1=ALU.mult,
                    accum_out=dsum[:, 6 * slot + j : 6 * slot + j + 1],
                )
            del gram_tiles[si]

        GRP = cfg["group"]
        GROUPS = [(0, 1), (1, 1), (2, 2), (4, 4), (8, 4), (12, 4)]
        if s_per_core != 16:
            GROUPS = [(s, 1) for s in range(s_per_core)]
        g_end = {st + sz - 1: gi for gi, (st, sz) in enumerate(GROUPS)}
        g_of = {}
        for gi, (st, sz) in enumerate(GROUPS):
            for s_ in range(st, st + sz):
                g_of[s_] = gi

        chain_sb = {}

        def emit_chain_a(g, dsum):
            """partition-reduce + W2 gather -> e_sb [1, 9*size]."""
            st, sz = GROUPS[g]
            p1t_ps = psum.tile([6 * GRP, 1], FP32, tag="p1t")
            nc.tensor.matmul(
                out=p1t_ps[: 6 * sz], lhsT=dsum[:, : 6 * sz], rhs=ones_k[:GCH]
            )
            p1t = small.tile([6 * GRP, 1], FP32, tag="p1t_sb")
            nc.scalar.copy(p1t[: 6 * sz], p1t_ps[: 6 * sz])
            e_ps = psum.tile([1, 9 * GRP], FP32, tag="e")
            nc.tensor.matmul(
                out=e_ps[:, : 9 * sz],
                lhsT=p1t[: 6 * sz],
                rhs=w2g[: 6 * sz, : 9 * sz],
            )
            e_sb = small.tile([1, 9 * GRP], FP32, tag="e_sb")
            nc.scalar.copy(e_sb[:, : 9 * sz], e_ps[:, : 9 * sz])
            chain_sb[g] = e_sb

        def emit_chain_b(g):
            """softmax rows + mb = gamma*att + I broadcast -> mb [P, 9*size]."""
            st, sz = GROUPS[g]
            n9 = 9 * sz
            e_sb = chain_sb.pop(g)
            e3 = e_sb[:, :n9].rearrange("p (sc d) -> p sc d", d=3)
            rmin = small.tile([1, 3 * GRP], FP32, tag="rmin")
            nc.vector.tensor_reduce(out=rmin[:, : 3 * sz], in_=e3, axis=AX.X, op=ALU.min)
            z = small.tile([1, 9 * GRP], FP32, tag="z")
            z3 = z[:, :n9].rearrange("p (sc d) -> p sc d", d=3)
            nc.vector.scalar_tensor_tensor(
                out=z3,
                in0=e3,
                scalar=-1.0,
                in1=_bcast(rmin[:, : 3 * sz], 3, 2),
                op0=ALU.mult,
                op1=ALU.add,
            )
            ex = small.tile([1, 9 * GRP], FP32, tag="ex")
            nc.scalar.activation(out=ex[:, :n9], in_=z[:, :n9], func=ACT.Exp)
            ex3 = ex[:, :n9].rearrange("p (sc d) -> p sc d", d=3)
            sm = small.tile([1, 3 * GRP], FP32, tag="sm")
            nc.vector.tensor_reduce(out=sm[:, : 3 * sz], in_=ex3, axis=AX.X, op=ALU.add)
            lnsm = small.tile([1, 3 * GRP], FP32, tag="lnsm")
            nc.scalar.activation(out=lnsm[:, : 3 * sz], in_=sm[:, : 3 * sz], func=ACT.Ln)
            w = small.tile([1, 9 * GRP], FP32, tag="w")
            nc.vector.scalar_tensor_tensor(
                out=w[:, :n9].rearrange("p (sc d) -> p sc d", d=3),
                in0=z3,
                scalar=1.0,
                in1=_bcast(lnsm[:, : 3 * sz], 3, 2),
                op0=ALU.mult,
                op1=ALU.subtract,
            )
            att = small.tile([1, 9 * GRP], FP32, tag="att")
            nc.scalar.activation(out=att[:, :n9], in_=w[:, :n9], func=ACT.Exp)
            mflat = small.tile([1, 9 * GRP], FP32, tag="mflat")
            nc.vector.scalar_tensor_tensor(
                out=mflat[:, :n9],
                in0=att[:, :n9],
                scalar=gamma_sb,
                in1=i9g[:, :n9],
                op0=ALU.mult,
                op1=ALU.add,
            )
            mb_ps = psum.tile([P, 9 * GRP], FP32, tag="mb")
            nc.tensor.matmul(out=mb_ps[:, :n9], lhsT=ones_b, rhs=mflat[:, :n9])
            mb = small.tile([P, 9 * GRP], FP32, tag="mb_sb")
            nc.scalar.copy(mb[:, :n9], mb_ps[:, :n9])
            mb_tiles[g] = mb

        u_tiles = {}

        def emit_muls(si, dve_heavy=False):
            """U_d[:, c, :] = x_d * mb[c,d] (9 scalar muls, split engines)."""
            xin = xin_tiles[si]
            g = g_of[si]
            mb = mb_tiles[g]
            off = 9 * (si - GROUPS[g][0])
            u0 = u_pool.tile([P, C, f], BF16, tag="u0")
            u1 = u_pool.tile([P, C, f], BF16, tag="u1")
            u2 = u_pool.tile([P, C, f], BF16, tag="u2")
            us = [u0, u1, u2]
            msc = MULS_SCALAR[:3] if dve_heavy else MULS_SCALAR
            mdv = [m for m in MULS_SCALAR + MULS_DVE if m not in msc]
            for d, c in msc:
                nc.scalar.mul(
                    us[d][:, c, :], xin[:, d, :],
                    mb[:, off + 3 * c + d : off + 3 * c + d + 1],
                )
            for d, c in mdv:
                nc.vector.tensor_scalar_mul(
                    us[d][:, c, :], xin[:, d, :],
                    mb[:, off + 3 * c + d : off + 3 * c + d + 1],
                )
            u_tiles[si] = us
            del xin_tiles[si]

        def emit_apply_tt(si):
            """out = U0 + U1 + U2 (c-fused TT adds) + store."""
            u0, u1, u2 = u_tiles[si]
            tsum = u_pool.tile([P, C, f], BF16, tag="tsum")
            nc.vector.tensor_tensor(out=tsum, in0=u0, in1=u1, op=ALU.add)
            outt = out_pool.tile([P, C, f], BF16, tag="outt")
            nc.vector.tensor_tensor(out=outt, in0=tsum, in1=u2, op=ALU.add)
            nc.sync.dma_start(out=o_d.ap()[si].rearrange("c p f -> p c f"), in_=outt)
            del u_tiles[si]

        # ---- software pipeline ----
        # U muls run one sample ahead of the TT adds; gram/extract run
        # AHEAD samples ahead; chains are split in two phases one
        # iteration apart; early groups are small so apply(0) starts fast.
        AHEAD = cfg["ahead"]
        lookahead = cfg["in_bufs"] - 1
        dsums = {}

        def group_dsum(g):
            if g not in dsums:
                dsums[g] = small.tile(
                    [GCH, 6 * GRP], FP32, tag="dsum", name=f"ds{g}"
                )
            return dsums[g]

        for si in range(min(lookahead, s_per_core)):
            emit_load(si)
        for si in range(min(AHEAD, s_per_core)):
            emit_gram(si)
            g = g_of[si]
            emit_extract(si, group_dsum(g), si - GROUPS[g][0])
            if si in g_end:
                emit_chain_a(g, dsums[g])
                emit_chain_b(g)
        emit_muls(0, dve_heavy=True)
        pending_b = []
        for s in range(s_per_core):
            if s + lookahead < s_per_core:
                emit_load(s + lookahead)
            s2 = s + AHEAD
            if s2 < s_per_core:
                emit_gram(s2)
            emit_apply_tt(s)
            if s + 1 < s_per_core:
                emit_muls(s + 1)
            if s2 < s_per_core:
                g = g_of[s2]
                emit_extract(s2, group_dsum(g), s2 - GROUPS[g][0])
            for g in pending_b:
                emit_chain_b(g)
            pending_b = []
            if s2 < s_per_core and s2 in g_end:
                emit_chain_a(g_of[s2], dsums[g_of[s2]])
                pending_b.append(g_of[s2])
        for g in pending_b:
            emit_chain_b(g)

    if split_waits:
        split_multi_waits(nc)
    return nc


def const_inputs():
    i9 = np.eye(3, dtype=np.float32).reshape(1, 9)
    w2 = np.zeros((6, 9), np.float32)
    for j, (a, b) in enumerate(PAIRS6):
        w2[j, 3 * a + b] = 1.0
        w2[j, 3 * b + a] = 1.0
    diagm = np.eye(P, dtype=np.float32)
    g = CFG["group"]
    w2g = np.kron(np.eye(g, dtype=np.float32), w2)
    i9g = np.tile(i9, (1, g))
    return {"i9c": i9, "diagm": diagm, "w2g": w2g, "i9g": i9g}


_NC_CACHE = {}


def _get_nc():
    key = "full"
    if key not in _NC_CACHE:
        _NC_CACHE[key] = build_kernel()
    return _NC_CACHE[key]


def kernel(x: np.ndarray, gamma: np.ndarray) -> np.ndarray:
    assert x.shape == (B, C, T, H, W) and x.dtype == np.float32
    nc = _get_nc()
    xs = np.ascontiguousarray(x).reshape(NCORES, S, C, P, F)
    g = np.asarray(gamma, dtype=np.float32).reshape(1, 1)
    cns = const_inputs()
    in_maps = [{"x": xs[i], "gamma": g, **cns} for i in range(NCORES)]
    res = run_bass_kernel_spmd(nc, in_maps, core_ids=list(range(NCORES)))
    out = np.stack(
        [np.asarray(res.results[i]["out"]).astype(np.float32) for i in range(NCORES)],
        axis=0,
    )
    return out.reshape(B, C, T, H, W)


def _install_ntff_hook():
    """The image's antenv lacks axon_hooks; synthesize it so
    run_bass_kernel_spmd(trace=True) can capture NTFF profiles."""
    import types

    try:
        from antenv.axon_hooks import get_axon_ntff_profile_hook  # noqa: F401

        return True
    except ImportError:
        pass
    try:
        import antenv

        mod = types.ModuleType("antenv.axon_hooks")
        _state = {"hook": None}

        def set_axon_ntff_profile_hook(h):
            _state["hook"] = h

        def get_axon_ntff_profile_hook():
            return _state["hook"]

        mod.set_axon_ntff_profile_hook = set_axon_ntff_profile_hook
        mod.get_axon_ntff_profile_hook = get_axon_ntff_profile_hook
        sys.modules["antenv.axon_hooks"] = mod
        antenv.axon_hooks = mod

        sys.path.insert(0, "/root/.axon_site")
        from trn_agent_boot.trn_boot import _ntff_profile_via_ctypes

        hook = _ntff_profile_via_ctypes("/opt/axon/libaxon_pjrt.so")
        if hook is None:
            return False
        set_axon_ntff_profile_hook(hook)
        return True
    except Exception as e:  # pragma: no cover
        print("ntff hook install failed:", e)
        return False


def profile_once(inputs):
    """Run with NTFF tracing; returns max per-core exec_time_ns."""
    _install_ntff_hook()
    x = np.asarray(inputs["x"])
    nc = _get_nc()
    xs = np.ascontiguousarray(x).reshape(NCORES, S, C, P, F)
    g = np.asarray(inputs["gamma"], dtype=np.float32).reshape(1, 1)
    cns = const_inputs()
    in_maps = [{"x": xs[i], "gamma": g, **cns} for i in range(NCORES)]
    res = run_bass_kernel_spmd(
        nc, in_maps, core_ids=list(range(NCORES)), trace=True
    )
    print("profile_json:", res.profile_json)
    print("exec_time_ns:", res.exec_time_ns, "mean:", res.mean_exec_time_ns)
    return res.exec_time_ns


if __name__ == "__main__":
    x = np.random.randn(B, C, T, H, W).astype(np.float32)
    gamma = np.zeros((1,), np.float32)
    y = kernel(x, gamma)
    print("ok", y.shape, float(np.abs(y - x).max()))
